# revision 4
# baseline (speedup 1.0000x reference)
"""ADMM-net 2D kernel for 8 TRN2 NeuronCores.

Math: in the reference, b stays exactly 0 and every stage is a linear map of
theta, so the whole 9-stage net collapses to theta = y @ M9 with

    M_0 = Phi,  M_{k+1} = M_k + (I - M_k Phi^T) S_k Phi,
    S_k = diag(1 / (rm + gamma_k)),  rm = rowwise ||Phi||^2.

On device we iterate the transposed form W_k = M_k^T (121x64, avoids any
large-tensor transposes in the setup):

    W_0 = Phi^T
    C_k = Phi @ W_k                       (64x64)
    D_k = S_k (I - C_k)                   (row scale)
    W_{k+1} = W_k + Phi^T @ D_k

then the single big matmul theta = y @ W9^T, tiled 128 rows at a time:
PE-transpose each y tile (128x64 -> 64x128), then matmul with M9 = W9^T
(64x121) replicated on both partition halves so two row-packed (K=64)
matmuls run concurrently in the PE array.

Sharding: pure data-parallel over the batch dim: 131072 rows -> 8 cores x
16384 rows. No collectives.
"""

import sys

if "/opt/trn_rl_repo" not in sys.path:
    sys.path.insert(0, "/opt/trn_rl_repo")

import numpy as np

B, M, N = 131072, 64, 121
STAGES = 9
NCORES = 8
BS = B // NCORES          # 16384 rows per core
TILES = BS // 128         # 128 row-tiles per core
CHUNK = 32                # row-tiles per DMA chunk
NCHUNKS = TILES // CHUNK  # 4

_cached = {}


def _build_nc():
    from concourse import bacc, mybir, tile

    f32 = mybir.dt.float32
    Alu = mybir.AluOpType

    nc = bacc.Bacc("TRN2", target_bir_lowering=False, debug=False)

    y_d = nc.dram_tensor("y", [BS, M], f32, kind="ExternalInput")
    phi_d = nc.dram_tensor("phi", [M, N], f32, kind="ExternalInput")
    gam_d = nc.dram_tensor("gam64", [M, STAGES], f32, kind="ExternalInput")
    id_d = nc.dram_tensor("ident", [128, 128], f32, kind="ExternalInput")
    out_d = nc.dram_tensor("out", [BS, N], f32, kind="ExternalOutput")

    with tile.TileContext(nc) as tc:
        with (
            tc.tile_pool(name="const", bufs=1) as constp,
            tc.tile_pool(name="setup", bufs=2) as setp,
            tc.tile_pool(name="psetup", bufs=1, space="PSUM") as psetp,
            tc.tile_pool(name="ypool", bufs=2) as ypool,
            tc.tile_pool(name="ytsb", bufs=3) as ytsbp,
            tc.tile_pool(name="opool", bufs=2) as opool,
            tc.tile_pool(name="ytps", bufs=2, space="PSUM") as ytpsp,
            tc.tile_pool(name="thps", bufs=3, space="PSUM") as thpsp,
        ):
            # ---- constants / small inputs ----
            ident_sb = constp.tile([128, 128], f32)
            nc.sync.dma_start(ident_sb[:], id_d[:])
            phi_sb = constp.tile([M, N], f32)
            nc.sync.dma_start(phi_sb[:], phi_d[:])
            gam_sb = constp.tile([M, STAGES], f32)
            nc.sync.dma_start(gam_sb[:], gam_d[:])

            # ---- setup: s = 1/(rm + gamma) ----
            sq = setp.tile([M, N], f32, tag="sq")
            nc.vector.tensor_tensor(sq[:], phi_sb[:], phi_sb[:], Alu.mult)
            rm = constp.tile([M, 1], f32)
            nc.vector.reduce_sum(rm[:], sq[:], axis=mybir.AxisListType.X)
            rg = setp.tile([M, STAGES], f32, tag="rg")
            nc.vector.tensor_scalar(rg[:], gam_sb[:], rm[:], None, Alu.add)
            s_sb = constp.tile([M, STAGES], f32)
            nc.vector.reciprocal(s_sb[:], rg[:])

            # ---- setup: W iteration ----
            phiT_ps = psetp.tile([N, M], f32, tag="tp")
            nc.tensor.transpose(phiT_ps[:], phi_sb[:], ident_sb[:M, :M])
            phiT_sb = constp.tile([N, M], f32)
            nc.vector.tensor_copy(phiT_sb[:], phiT_ps[:])
            w_sb = setp.tile([N, M], f32, tag="w")
            nc.vector.tensor_copy(w_sb[:], phiT_ps[:])

            for k in range(STAGES):
                c_ps = psetp.tile([M, M], f32, tag="c")
                nc.tensor.matmul(c_ps[:], phiT_sb[:], w_sb[:])  # Phi @ W_k
                d_sb = setp.tile([M, M], f32, tag="d")
                nc.vector.tensor_tensor(
                    d_sb[:], ident_sb[:M, :M], c_ps[:], Alu.subtract
                )
                nc.vector.tensor_scalar(
                    d_sb[:], d_sb[:], s_sb[:, k : k + 1], None, Alu.mult
                )
                wn_ps = psetp.tile([N, M], f32, tag="wn")
                nc.tensor.matmul(wn_ps[:], phi_sb[:], d_sb[:])  # Phi^T @ D
                w_new = setp.tile([N, M], f32, tag="w")
                nc.vector.tensor_tensor(w_new[:], w_sb[:], wn_ps[:], Alu.add)
                w_sb = w_new

            # ---- M9 = W9^T, replicated on both partition halves ----
            # Place W9 twice side-by-side [121, 128]; one transpose then
            # yields M9 stacked at partitions 0-63 and 64-127.
            w2_sb = setp.tile([N, 128], f32, tag="w2")
            nc.vector.tensor_copy(w2_sb[:, :M], w_sb[:])
            nc.vector.tensor_copy(w2_sb[:, M:], w_sb[:])
            m9_ps = psetp.tile([128, N], f32, tag="tp")
            nc.tensor.transpose(m9_ps[:], w2_sb[:], ident_sb[:N, :N])
            m9_sb = constp.tile([128, N], f32)
            nc.vector.tensor_copy(m9_sb[:], m9_ps[:])

            # ---- main loop: theta = y @ M9, 128-row tiles ----
            for c in range(NCHUNKS):
                r0 = c * CHUNK * 128
                r1 = (c + 1) * CHUNK * 128
                y_sb = ypool.tile([128, CHUNK, M], f32, tag="y")
                nc.sync.dma_start(
                    y_sb[:], y_d[r0:r1, :].rearrange("(t p) m -> p t m", p=128)
                )
                th_sb = opool.tile([128, CHUNK, N], f32, tag="th")
                for g in range(CHUNK // 8):
                    yt_ps = ytpsp.tile([128, 512], f32, tag="ytp")
                    # One [128,128] transpose handles a PAIR of row-tiles:
                    # tile 2i lands at psum partitions 0-63, tile 2i+1 at
                    # 64-127 (base partition stays 0, as walrus requires).
                    for i in range(4):
                        t = g * 8 + 2 * i
                        nc.tensor.transpose(
                            yt_ps[:, i * 128 : (i + 1) * 128],
                            y_sb[:, t : t + 2, :],
                            ident_sb[:],
                        )
                    yt_sb = ytsbp.tile([128, 512], f32, tag="yts")
                    nc.vector.tensor_copy(yt_sb[:], yt_ps[:])
                    # Adjacent matmuls alternate PE row-groups (concurrent)
                    # and MUST land in different PSUM banks: even tiles ->
                    # thA, odd tiles -> thB.
                    thA = thpsp.tile([128, 4, N], f32, tag="thp")
                    thB = thpsp.tile([128, 4, N], f32, tag="thp")
                    for i in range(4):
                        slot = 128 * i
                        nc.tensor.matmul(
                            thA[:, i, :],
                            yt_sb[0:64, slot : slot + 128],
                            m9_sb[0:64, :],
                            tile_position=(0, 0),
                        )
                        nc.tensor.matmul(
                            thB[:, i, :],
                            yt_sb[64:128, slot : slot + 128],
                            m9_sb[64:128, :],
                            tile_position=(64, 0),
                        )
                    tbase = g * 8
                    nc.vector.tensor_copy(
                        th_sb[:, tbase : tbase + 8 : 2, :], thA[:]
                    )
                    nc.scalar.copy(
                        th_sb[:, tbase + 1 : tbase + 8 : 2, :], thB[:]
                    )
                nc.sync.dma_start(
                    out_d[r0:r1, :].rearrange("(t p) n -> p t n", p=128), th_sb[:]
                )

    nc.compile()
    return nc


def _get_nc():
    if "nc" not in _cached:
        _cached["nc"] = _build_nc()
    return _cached["nc"]


def kernel(y, Phi, gammas):
    from concourse.bass_utils import run_bass_kernel_spmd

    y = np.ascontiguousarray(y, dtype=np.float32)
    phi = np.ascontiguousarray(Phi, dtype=np.float32)
    gam64 = np.ascontiguousarray(
        np.broadcast_to(np.asarray(gammas, dtype=np.float32).reshape(1, STAGES), (M, STAGES))
    )
    ident = np.eye(128, dtype=np.float32)

    nc = _get_nc()
    in_maps = [
        {
            "y": np.ascontiguousarray(y[i * BS : (i + 1) * BS]),
            "phi": phi,
            "gam64": gam64,
            "ident": ident,
        }
        for i in range(NCORES)
    ]
    res = run_bass_kernel_spmd(nc, in_maps, core_ids=list(range(NCORES)))
    _cached["last_run"] = res
    return np.concatenate([res.results[i]["out"] for i in range(NCORES)], axis=0)


# revision 6
# speedup vs baseline: 1.5776x; 1.5776x over previous
"""ADMM-net 2D kernel for 8 TRN2 NeuronCores.

Math: in the reference, b stays exactly 0 and every stage is a linear map of
theta, so the whole 9-stage net collapses to theta = y @ M9 with

    M_0 = Phi,  M_{k+1} = M_k + (I - M_k Phi^T) S_k Phi,
    S_k = diag(1 / (rm + gamma_k)),  rm = rowwise ||Phi||^2.

On device we iterate the transposed form W_k = M_k^T (121x64, avoids any
large-tensor transposes in the setup):

    W_0 = Phi^T
    C_k = Phi @ W_k                       (64x64)
    D_k = S_k (I - C_k)                   (row scale)
    W_{k+1} = W_k + Phi^T @ D_k

then the single big matmul theta = y @ W9^T, tiled 128 rows at a time:
PE-transpose each y tile (128x64 -> 64x128), then matmul with M9 = W9^T
(64x121) replicated on both partition halves so two row-packed (K=64)
matmuls run concurrently in the PE array.

Sharding: pure data-parallel over the batch dim: 131072 rows -> 8 cores x
16384 rows. No collectives.
"""

import sys

if "/opt/trn_rl_repo" not in sys.path:
    sys.path.insert(0, "/opt/trn_rl_repo")

import numpy as np

B, M, N = 131072, 64, 121
STAGES = 9
NCORES = 8
BS = B // NCORES          # 16384 rows per core
TILES = BS // 128         # 128 row-tiles per core
CHUNK = 32                # row-tiles per DMA chunk
NCHUNKS = TILES // CHUNK  # 4

_cached = {}


def _build_nc():
    from concourse import bacc, mybir, tile

    f32 = mybir.dt.float32
    Alu = mybir.AluOpType

    nc = bacc.Bacc("TRN2", target_bir_lowering=False, debug=False)

    y_d = nc.dram_tensor("y", [BS, M], f32, kind="ExternalInput")
    phi_d = nc.dram_tensor("phi", [M, N], f32, kind="ExternalInput")
    gam_d = nc.dram_tensor("gam64", [M, STAGES], f32, kind="ExternalInput")
    id_d = nc.dram_tensor("ident", [128, 128], f32, kind="ExternalInput")
    out_d = nc.dram_tensor("out", [BS, N], f32, kind="ExternalOutput")

    with tile.TileContext(nc) as tc:
        with (
            tc.tile_pool(name="const", bufs=1) as constp,
            tc.tile_pool(name="setup", bufs=2) as setp,
            tc.tile_pool(name="psetup", bufs=1, space="PSUM") as psetp,
            tc.tile_pool(name="ypool", bufs=2) as ypool,
            tc.tile_pool(name="ytsb", bufs=3) as ytsbp,
            tc.tile_pool(name="opool", bufs=2) as opool,
            tc.tile_pool(name="ytps", bufs=2, space="PSUM") as ytpsp,
            tc.tile_pool(name="thps", bufs=3, space="PSUM") as thpsp,
        ):
            # ---- constants / small inputs ----
            ident_sb = constp.tile([128, 128], f32)
            nc.sync.dma_start(ident_sb[:], id_d[:])
            phi_sb = constp.tile([M, N], f32)
            nc.sync.dma_start(phi_sb[:], phi_d[:])
            gam_sb = constp.tile([M, STAGES], f32)
            nc.sync.dma_start(gam_sb[:], gam_d[:])

            # ---- setup: s = 1/(rm + gamma) ----
            sq = setp.tile([M, N], f32, tag="sq")
            nc.vector.tensor_tensor(sq[:], phi_sb[:], phi_sb[:], Alu.mult)
            rm = constp.tile([M, 1], f32)
            nc.vector.reduce_sum(rm[:], sq[:], axis=mybir.AxisListType.X)
            rg = setp.tile([M, STAGES], f32, tag="rg")
            nc.vector.tensor_scalar(rg[:], gam_sb[:], rm[:], None, Alu.add)
            s_sb = constp.tile([M, STAGES], f32)
            nc.vector.reciprocal(s_sb[:], rg[:])

            # ---- setup: W iteration ----
            phiT_ps = psetp.tile([N, M], f32, tag="tp")
            nc.tensor.transpose(phiT_ps[:], phi_sb[:], ident_sb[:M, :M])
            phiT_sb = constp.tile([N, M], f32)
            nc.vector.tensor_copy(phiT_sb[:], phiT_ps[:])
            w_sb = setp.tile([N, M], f32, tag="w")
            nc.vector.tensor_copy(w_sb[:], phiT_ps[:])

            for k in range(STAGES):
                c_ps = psetp.tile([M, M], f32, tag="c")
                nc.tensor.matmul(c_ps[:], phiT_sb[:], w_sb[:])  # Phi @ W_k
                d_sb = setp.tile([M, M], f32, tag="d")
                nc.vector.tensor_tensor(
                    d_sb[:], ident_sb[:M, :M], c_ps[:], Alu.subtract
                )
                nc.vector.tensor_scalar(
                    d_sb[:], d_sb[:], s_sb[:, k : k + 1], None, Alu.mult
                )
                wn_ps = psetp.tile([N, M], f32, tag="wn")
                nc.tensor.matmul(wn_ps[:], phi_sb[:], d_sb[:])  # Phi^T @ D
                w_new = setp.tile([N, M], f32, tag="w")
                nc.vector.tensor_tensor(w_new[:], w_sb[:], wn_ps[:], Alu.add)
                w_sb = w_new

            # ---- M9 = W9^T, replicated on both partition halves ----
            # Place W9 twice side-by-side [121, 128]; one transpose then
            # yields M9 stacked at partitions 0-63 and 64-127.
            w2_sb = setp.tile([N, 128], f32, tag="w2")
            nc.vector.tensor_copy(w2_sb[:, :M], w_sb[:])
            nc.vector.tensor_copy(w2_sb[:, M:], w_sb[:])
            m9_ps = psetp.tile([128, N], f32, tag="tp")
            nc.tensor.transpose(m9_ps[:], w2_sb[:], ident_sb[:N, :N])
            m9_sb = constp.tile([128, N], f32)
            nc.vector.tensor_copy(m9_sb[:], m9_ps[:])

            # ---- main loop: theta = y @ M9, 128-row tiles ----
            # Row-tile c is the STRIDED row set {p*128 + c : p in 0..127}
            # (a pure permutation of rows). This makes each partition's DMA
            # data one long contiguous DRAM run (8KB in / 15.5KB out) instead
            # of per-row 256B/484B descriptors.
            y_v = y_d[:].rearrange("(p c) m -> p c m", c=TILES)
            out_v = out_d[:].rearrange("(p c) n -> p c n", c=TILES)
            for c in range(NCHUNKS):
                c0 = c * CHUNK
                y_sb = ypool.tile([128, CHUNK, M], f32, tag="y")
                nc.sync.dma_start(y_sb[:], y_v[:, c0 : c0 + CHUNK, :])
                th_sb = opool.tile([128, CHUNK, N], f32, tag="th")
                for g in range(CHUNK // 8):
                    yt_ps = ytpsp.tile([128, 512], f32, tag="ytp")
                    # One [128,128] transpose handles a PAIR of row-tiles:
                    # tile 2i lands at psum partitions 0-63, tile 2i+1 at
                    # 64-127 (base partition stays 0, as walrus requires).
                    for i in range(4):
                        t = g * 8 + 2 * i
                        nc.tensor.transpose(
                            yt_ps[:, i * 128 : (i + 1) * 128],
                            y_sb[:, t : t + 2, :],
                            ident_sb[:],
                        )
                    yt_sb = ytsbp.tile([128, 512], f32, tag="yts")
                    nc.vector.tensor_copy(yt_sb[:], yt_ps[:])
                    # Adjacent matmuls alternate PE row-groups (concurrent)
                    # and MUST land in different PSUM banks: even tiles ->
                    # thA, odd tiles -> thB.
                    thA = thpsp.tile([128, 4, N], f32, tag="thp")
                    thB = thpsp.tile([128, 4, N], f32, tag="thp")
                    for i in range(4):
                        slot = 128 * i
                        nc.tensor.matmul(
                            thA[:, i, :],
                            yt_sb[0:64, slot : slot + 128],
                            m9_sb[0:64, :],
                            tile_position=(0, 0),
                        )
                        nc.tensor.matmul(
                            thB[:, i, :],
                            yt_sb[64:128, slot : slot + 128],
                            m9_sb[64:128, :],
                            tile_position=(64, 0),
                        )
                    tbase = g * 8
                    nc.vector.tensor_copy(
                        th_sb[:, tbase : tbase + 8 : 2, :], thA[:]
                    )
                    nc.scalar.copy(
                        th_sb[:, tbase + 1 : tbase + 8 : 2, :], thB[:]
                    )
                nc.sync.dma_start(out_v[:, c0 : c0 + CHUNK, :], th_sb[:])

    nc.compile()
    return nc


def _get_nc():
    if "nc" not in _cached:
        _cached["nc"] = _build_nc()
    return _cached["nc"]


def kernel(y, Phi, gammas):
    from concourse.bass_utils import run_bass_kernel_spmd

    y = np.ascontiguousarray(y, dtype=np.float32)
    phi = np.ascontiguousarray(Phi, dtype=np.float32)
    gam64 = np.ascontiguousarray(
        np.broadcast_to(np.asarray(gammas, dtype=np.float32).reshape(1, STAGES), (M, STAGES))
    )
    ident = np.eye(128, dtype=np.float32)

    nc = _get_nc()
    in_maps = [
        {
            "y": np.ascontiguousarray(y[i * BS : (i + 1) * BS]),
            "phi": phi,
            "gam64": gam64,
            "ident": ident,
        }
        for i in range(NCORES)
    ]
    res = run_bass_kernel_spmd(nc, in_maps, core_ids=list(range(NCORES)))
    _cached["last_run"] = res
    return np.concatenate([res.results[i]["out"] for i in range(NCORES)], axis=0)


# revision 9
# speedup vs baseline: 1.7329x; 1.0985x over previous
"""ADMM-net 2D kernel for 8 TRN2 NeuronCores.

Math: in the reference, b stays exactly 0 and every stage is a linear map of
theta, so the whole 9-stage net collapses to theta = y @ M9 with

    M_0 = Phi,  M_{k+1} = M_k + (I - M_k Phi^T) S_k Phi,
    S_k = diag(1 / (rm + gamma_k)),  rm = rowwise ||Phi||^2.

On device we iterate the transposed form W_k = M_k^T (121x64, avoids any
large-tensor transposes in the setup):

    W_0 = Phi^T
    C_k = Phi @ W_k                       (64x64)
    D_k = S_k (I - C_k)                   (row scale)
    W_{k+1} = W_k + Phi^T @ D_k

then the single big matmul theta = y @ W9^T, tiled 128 rows at a time:
PE-transpose each y tile (128x64 -> 64x128), then matmul with M9 = W9^T
(64x121) replicated on both partition halves so two row-packed (K=64)
matmuls run concurrently in the PE array.

Sharding: pure data-parallel over the batch dim: 131072 rows -> 8 cores x
16384 rows. No collectives.
"""

import sys

if "/opt/trn_rl_repo" not in sys.path:
    sys.path.insert(0, "/opt/trn_rl_repo")

import numpy as np

B, M, N = 131072, 64, 121
STAGES = 9
NCORES = 8
BS = B // NCORES          # 16384 rows per core
TILES = BS // 128         # 128 row-tiles per core
CHUNK = 32                # row-tiles per DMA chunk
NCHUNKS = TILES // CHUNK  # 4

_cached = {}


def _build_nc():
    from concourse import bacc, mybir, tile

    f32 = mybir.dt.float32
    f16 = mybir.dt.float16
    Alu = mybir.AluOpType
    SCALE = float(2.0 ** 36)

    nc = bacc.Bacc("TRN2", target_bir_lowering=False, debug=False)

    y_d = nc.dram_tensor("y", [BS, M], f32, kind="ExternalInput")
    phi_d = nc.dram_tensor("phi", [M, N], f32, kind="ExternalInput")
    gam_d = nc.dram_tensor("gam64", [M, STAGES], f32, kind="ExternalInput")
    id_d = nc.dram_tensor("ident", [128, 128], f32, kind="ExternalInput")
    idh_d = nc.dram_tensor("identh", [128, 128], f16, kind="ExternalInput")
    out_d = nc.dram_tensor("out", [BS, N], f32, kind="ExternalOutput")

    with tile.TileContext(nc) as tc:
        with (
            tc.tile_pool(name="const", bufs=1) as constp,
            tc.tile_pool(name="setup", bufs=2) as setp,
            tc.tile_pool(name="psetup", bufs=1, space="PSUM") as psetp,
            tc.tile_pool(name="ypool", bufs=2) as ypool,
            tc.tile_pool(name="ytsb", bufs=3) as ytsbp,
            tc.tile_pool(name="opool", bufs=2) as opool,
            tc.tile_pool(name="ytps", bufs=2, space="PSUM") as ytpsp,
            tc.tile_pool(name="thps", bufs=3, space="PSUM") as thpsp,
        ):
            # ---- constants / small inputs ----
            ident_sb = constp.tile([128, 128], f32)
            nc.sync.dma_start(ident_sb[:], id_d[:])
            identh_sb = constp.tile([128, 128], f16)
            nc.sync.dma_start(identh_sb[:], idh_d[:])
            phi_sb = constp.tile([M, N], f32)
            nc.sync.dma_start(phi_sb[:], phi_d[:])
            gam_sb = constp.tile([M, STAGES], f32)
            nc.sync.dma_start(gam_sb[:], gam_d[:])

            # ---- setup: s = 1/(rm + gamma) ----
            sq = setp.tile([M, N], f32, tag="sq")
            nc.vector.tensor_tensor(sq[:], phi_sb[:], phi_sb[:], Alu.mult)
            rm = constp.tile([M, 1], f32)
            nc.vector.reduce_sum(rm[:], sq[:], axis=mybir.AxisListType.X)
            rg = setp.tile([M, STAGES], f32, tag="rg")
            nc.vector.tensor_scalar(rg[:], gam_sb[:], rm[:], None, Alu.add)
            s_sb = constp.tile([M, STAGES], f32)
            nc.vector.reciprocal(s_sb[:], rg[:])

            # ---- setup: W iteration ----
            phiT_ps = psetp.tile([N, M], f32, tag="tp")
            nc.tensor.transpose(phiT_ps[:], phi_sb[:], ident_sb[:M, :M])
            phiT_sb = constp.tile([N, M], f32)
            nc.vector.tensor_copy(phiT_sb[:], phiT_ps[:])
            w_sb = setp.tile([N, M], f32, tag="w")
            nc.vector.tensor_copy(w_sb[:], phiT_ps[:])

            for k in range(STAGES):
                c_ps = psetp.tile([M, M], f32, tag="c")
                nc.tensor.matmul(c_ps[:], phiT_sb[:], w_sb[:])  # Phi @ W_k
                d_sb = setp.tile([M, M], f32, tag="d")
                nc.vector.tensor_tensor(
                    d_sb[:], ident_sb[:M, :M], c_ps[:], Alu.subtract
                )
                nc.vector.tensor_scalar(
                    d_sb[:], d_sb[:], s_sb[:, k : k + 1], None, Alu.mult
                )
                wn_ps = psetp.tile([N, M], f32, tag="wn")
                nc.tensor.matmul(wn_ps[:], phi_sb[:], d_sb[:])  # Phi^T @ D
                w_new = setp.tile([N, M], f32, tag="w")
                nc.vector.tensor_tensor(w_new[:], w_sb[:], wn_ps[:], Alu.add)
                w_sb = w_new

            # ---- M9 = W9^T, replicated on both partition halves ----
            # Place W9 twice side-by-side [121, 128]; one transpose then
            # yields M9 stacked at partitions 0-63 and 64-127.
            w2_sb = setp.tile([N, 128], f32, tag="w2")
            nc.vector.tensor_copy(w2_sb[:, :M], w_sb[:])
            nc.vector.tensor_copy(w2_sb[:, M:], w_sb[:])
            m9_ps = psetp.tile([128, N], f32, tag="tp")
            nc.tensor.transpose(m9_ps[:], w2_sb[:], ident_sb[:N, :N])
            m9h_sb = constp.tile([128, N], f16)
            nc.vector.tensor_scalar(
                m9h_sb[:], m9_ps[:], 1.0 / SCALE, None, Alu.mult
            )

            # ---- main loop: theta = y @ M9, 128-row tiles ----
            # Row-tile c is the STRIDED row set {p*128 + c : p in 0..127}
            # (a pure permutation of rows). This makes each partition's DMA
            # data one long contiguous DRAM run (8KB in / 15.5KB out) instead
            # of per-row 256B/484B descriptors.
            y_v = y_d[:].rearrange("(p c) m -> p c m", c=TILES)
            out_v = out_d[:].rearrange("(p c) n -> p c n", c=TILES)
            for c in range(NCHUNKS):
                c0 = c * CHUNK
                y_sb = ypool.tile([128, CHUNK, M], f16, tag="y")
                nc.gpsimd.dma_start(y_sb[:], y_v[:, c0 : c0 + CHUNK, :])
                th_sb = opool.tile([128, CHUNK, N], f32, tag="th")
                for g in range(CHUNK // 8):
                    yt_ps = ytpsp.tile([128, 512], f16, tag="ytp")
                    # One [128,128] transpose handles a PAIR of row-tiles:
                    # tile 2i lands at psum partitions 0-63, tile 2i+1 at
                    # 64-127 (base partition stays 0, as walrus requires).
                    for i in range(4):
                        t = g * 8 + 2 * i
                        nc.tensor.transpose(
                            yt_ps[:, i * 128 : (i + 1) * 128],
                            y_sb[:, t : t + 2, :],
                            identh_sb[:],
                        )
                    yt_sb = ytsbp.tile([128, 512], f16, tag="yts")
                    nc.vector.tensor_copy(yt_sb[:], yt_ps[:])
                    # Adjacent matmuls alternate PE row-groups (concurrent)
                    # and MUST land in different PSUM banks: even tiles ->
                    # thA, odd tiles -> thB.
                    thA = thpsp.tile([128, 4, N], f32, tag="thp")
                    thB = thpsp.tile([128, 4, N], f32, tag="thp")
                    for i in range(4):
                        slot = 128 * i
                        nc.tensor.matmul(
                            thA[:, i, :],
                            yt_sb[0:64, slot : slot + 128],
                            m9h_sb[0:64, :],
                            tile_position=(0, 0),
                        )
                        nc.tensor.matmul(
                            thB[:, i, :],
                            yt_sb[64:128, slot : slot + 128],
                            m9h_sb[64:128, :],
                            tile_position=(64, 0),
                        )
                    tbase = g * 8
                    nc.vector.tensor_scalar(
                        th_sb[:, tbase : tbase + 8 : 2, :], thA[:],
                        SCALE, None, Alu.mult,
                    )
                    nc.scalar.mul(
                        th_sb[:, tbase + 1 : tbase + 8 : 2, :], thB[:], SCALE
                    )
                nc.sync.dma_start(out_v[:, c0 : c0 + CHUNK, :], th_sb[:])

    nc.compile()
    return nc


def _get_nc():
    if "nc" not in _cached:
        _cached["nc"] = _build_nc()
    return _cached["nc"]


def kernel(y, Phi, gammas):
    from concourse.bass_utils import run_bass_kernel_spmd

    y = np.ascontiguousarray(y, dtype=np.float32)
    phi = np.ascontiguousarray(Phi, dtype=np.float32)
    gam64 = np.ascontiguousarray(
        np.broadcast_to(np.asarray(gammas, dtype=np.float32).reshape(1, STAGES), (M, STAGES))
    )
    ident = np.eye(128, dtype=np.float32)
    identh = np.eye(128, dtype=np.float16)

    nc = _get_nc()
    in_maps = [
        {
            "y": np.ascontiguousarray(y[i * BS : (i + 1) * BS]),
            "phi": phi,
            "gam64": gam64,
            "ident": ident,
            "identh": identh,
        }
        for i in range(NCORES)
    ]
    res = run_bass_kernel_spmd(nc, in_maps, core_ids=list(range(NCORES)))
    _cached["last_run"] = res
    return np.concatenate([res.results[i]["out"] for i in range(NCORES)], axis=0)


# revision 11
# speedup vs baseline: 1.7348x; 1.0011x over previous
"""ADMM-net 2D kernel for 8 TRN2 NeuronCores.

Math: in the reference, b stays exactly 0 and every stage is a linear map of
theta, so the whole 9-stage net collapses to theta = y @ M9 with

    M_0 = Phi,  M_{k+1} = M_k + (I - M_k Phi^T) S_k Phi,
    S_k = diag(1 / (rm + gamma_k)),  rm = rowwise ||Phi||^2.

On device we iterate the transposed form W_k = M_k^T (121x64, avoids any
large-tensor transposes in the setup):

    W_0 = Phi^T
    C_k = Phi @ W_k                       (64x64)
    D_k = S_k (I - C_k)                   (row scale)
    W_{k+1} = W_k + Phi^T @ D_k

then the single big matmul theta = y @ W9^T, tiled 128 rows at a time:
PE-transpose each y tile (128x64 -> 64x128), then matmul with M9 = W9^T
(64x121) replicated on both partition halves so two row-packed (K=64)
matmuls run concurrently in the PE array.

Sharding: pure data-parallel over the batch dim: 131072 rows -> 8 cores x
16384 rows. No collectives.
"""

import sys

if "/opt/trn_rl_repo" not in sys.path:
    sys.path.insert(0, "/opt/trn_rl_repo")

import numpy as np

B, M, N = 131072, 64, 121
STAGES = 9
NCORES = 8
BS = B // NCORES          # 16384 rows per core
TILES = BS // 128         # 128 row-tiles per core
CHUNK = 16                # row-tiles per DMA chunk
NCHUNKS = TILES // CHUNK  # 4

_cached = {}


def _build_nc():
    from concourse import bacc, mybir, tile

    f32 = mybir.dt.float32
    f16 = mybir.dt.float16
    Alu = mybir.AluOpType
    SCALE = float(2.0 ** 36)

    nc = bacc.Bacc("TRN2", target_bir_lowering=False, debug=False)

    y_d = nc.dram_tensor("y", [BS, M], f32, kind="ExternalInput")
    blob_d = nc.dram_tensor("blob", [128, 128 + 64 + N + STAGES], f32,
                            kind="ExternalInput")
    out_d = nc.dram_tensor("out", [BS, N], f32, kind="ExternalOutput")

    with tile.TileContext(nc) as tc:
        with (
            tc.tile_pool(name="const", bufs=1) as constp,
            tc.tile_pool(name="setup", bufs=2) as setp,
            tc.tile_pool(name="psetup", bufs=1, space="PSUM") as psetp,
            tc.tile_pool(name="ypool", bufs=2) as ypool,
            tc.tile_pool(name="ytsb", bufs=3) as ytsbp,
            tc.tile_pool(name="opool", bufs=2) as opool,
            tc.tile_pool(name="ytps", bufs=2, space="PSUM") as ytpsp,
            tc.tile_pool(name="thps", bufs=3, space="PSUM") as thpsp,
        ):
            # ---- constants / small inputs: ONE packed DMA ----
            blob_sb = constp.tile([128, 128 + 64 + N + STAGES], f32)
            nc.sync.dma_start(blob_sb[:], blob_d[:])
            ident_sb = blob_sb[:, 0:128]
            identh_sb = blob_sb[:, 128:192].bitcast(f16)
            phi_sb = blob_sb[:M, 192 : 192 + N]
            gam_sb = blob_sb[:M, 192 + N : 192 + N + STAGES]

            # ---- setup: s = 1/(rm + gamma) ----
            sq = setp.tile([M, N], f32, tag="sq")
            nc.vector.tensor_tensor(sq[:], phi_sb, phi_sb, Alu.mult)
            rm = constp.tile([M, 1], f32)
            nc.vector.reduce_sum(rm[:], sq[:], axis=mybir.AxisListType.X)
            rg = setp.tile([M, STAGES], f32, tag="rg")
            nc.vector.tensor_scalar(rg[:], gam_sb, rm[:], None, Alu.add)
            s_sb = constp.tile([M, STAGES], f32)
            nc.vector.reciprocal(s_sb[:], rg[:])

            # ---- setup: W iteration ----
            phiT_ps = psetp.tile([N, M], f32, tag="tp")
            nc.tensor.transpose(phiT_ps[:], phi_sb, ident_sb[:M, :M])
            phiT_sb = constp.tile([N, M], f32)
            nc.vector.tensor_copy(phiT_sb[:], phiT_ps[:])
            w_sb = setp.tile([N, M], f32, tag="w")
            nc.vector.tensor_copy(w_sb[:], phiT_ps[:])

            for k in range(STAGES):
                c_ps = psetp.tile([M, M], f32, tag="c")
                nc.tensor.matmul(c_ps[:], phiT_sb[:], w_sb[:])  # Phi @ W_k
                d_sb = setp.tile([M, M], f32, tag="d")
                nc.vector.tensor_tensor(
                    d_sb[:], ident_sb[:M, :M], c_ps[:], Alu.subtract
                )
                nc.vector.tensor_scalar(
                    d_sb[:], d_sb[:], s_sb[:, k : k + 1], None, Alu.mult
                )
                wn_ps = psetp.tile([N, M], f32, tag="wn")
                nc.tensor.matmul(wn_ps[:], phi_sb, d_sb[:])  # Phi^T @ D
                w_new = setp.tile([N, M], f32, tag="w")
                nc.vector.tensor_tensor(w_new[:], w_sb[:], wn_ps[:], Alu.add)
                w_sb = w_new

            # ---- M9 = W9^T, replicated on both partition halves ----
            # Place W9 twice side-by-side [121, 128]; one transpose then
            # yields M9 stacked at partitions 0-63 and 64-127.
            w2_sb = setp.tile([N, 128], f32, tag="w2")
            nc.vector.tensor_copy(w2_sb[:, :M], w_sb[:])
            nc.vector.tensor_copy(w2_sb[:, M:], w_sb[:])
            m9_ps = psetp.tile([128, N], f32, tag="tp")
            nc.tensor.transpose(m9_ps[:], w2_sb[:], ident_sb[:N, :N])
            m9h_sb = constp.tile([128, N], f16)
            nc.vector.tensor_scalar(
                m9h_sb[:], m9_ps[:], 1.0 / SCALE, None, Alu.mult
            )

            # ---- main loop: theta = y @ M9, 128-row tiles ----
            # Row-tile c is the STRIDED row set {p*128 + c : p in 0..127}
            # (a pure permutation of rows). This makes each partition's DMA
            # data one long contiguous DRAM run (8KB in / 15.5KB out) instead
            # of per-row 256B/484B descriptors.
            y_v = y_d[:].rearrange("(p c) m -> p c m", c=TILES)
            out_v = out_d[:].rearrange("(p c) n -> p c n", c=TILES)
            for c in range(NCHUNKS):
                c0 = c * CHUNK
                y_sb = ypool.tile([128, CHUNK, M], f16, tag="y")
                nc.gpsimd.dma_start(y_sb[:], y_v[:, c0 : c0 + CHUNK, :])
                th_sb = opool.tile([128, CHUNK, N], f32, tag="th")
                for g in range(CHUNK // 8):
                    yt_ps = ytpsp.tile([128, 512], f16, tag="ytp")
                    # One [128,128] transpose handles a PAIR of row-tiles:
                    # tile 2i lands at psum partitions 0-63, tile 2i+1 at
                    # 64-127 (base partition stays 0, as walrus requires).
                    for i in range(4):
                        t = g * 8 + 2 * i
                        nc.tensor.transpose(
                            yt_ps[:, i * 128 : (i + 1) * 128],
                            y_sb[:, t : t + 2, :],
                            identh_sb,
                        )
                    yt_sb = ytsbp.tile([128, 512], f16, tag="yts")
                    nc.vector.tensor_copy(yt_sb[:], yt_ps[:])
                    # Adjacent matmuls alternate PE row-groups (concurrent)
                    # and MUST land in different PSUM banks: even tiles ->
                    # thA, odd tiles -> thB.
                    thA = thpsp.tile([128, 4, N], f32, tag="thp")
                    thB = thpsp.tile([128, 4, N], f32, tag="thp")
                    for i in range(4):
                        slot = 128 * i
                        nc.tensor.matmul(
                            thA[:, i, :],
                            yt_sb[0:64, slot : slot + 128],
                            m9h_sb[0:64, :],
                            tile_position=(0, 0),
                        )
                        nc.tensor.matmul(
                            thB[:, i, :],
                            yt_sb[64:128, slot : slot + 128],
                            m9h_sb[64:128, :],
                            tile_position=(64, 0),
                        )
                    tbase = g * 8
                    nc.vector.tensor_scalar(
                        th_sb[:, tbase : tbase + 8 : 2, :], thA[:],
                        SCALE, None, Alu.mult,
                    )
                    nc.scalar.mul(
                        th_sb[:, tbase + 1 : tbase + 8 : 2, :], thB[:], SCALE
                    )
                nc.sync.dma_start(out_v[:, c0 : c0 + CHUNK, :], th_sb[:])

    nc.compile()
    return nc


def _get_nc():
    if "nc" not in _cached:
        _cached["nc"] = _build_nc()
    return _cached["nc"]


def kernel(y, Phi, gammas):
    from concourse.bass_utils import run_bass_kernel_spmd

    y = np.ascontiguousarray(y, dtype=np.float32)
    phi = np.asarray(Phi, dtype=np.float32)
    gam = np.asarray(gammas, dtype=np.float32).reshape(1, STAGES)

    blob = np.zeros((128, 128 + 64 + N + STAGES), dtype=np.float32)
    blob[:, 0:128] = np.eye(128, dtype=np.float32)
    blob[:, 128:192] = np.eye(128, dtype=np.float16).view(np.float32)
    blob[:M, 192 : 192 + N] = phi
    blob[:M, 192 + N : 192 + N + STAGES] = np.broadcast_to(gam, (M, STAGES))

    nc = _get_nc()
    in_maps = [
        {
            "y": np.ascontiguousarray(y[i * BS : (i + 1) * BS]),
            "blob": blob,
        }
        for i in range(NCORES)
    ]
    res = run_bass_kernel_spmd(nc, in_maps, core_ids=list(range(NCORES)))
    _cached["last_run"] = res
    return np.concatenate([res.results[i]["out"] for i in range(NCORES)], axis=0)


# revision 12
# speedup vs baseline: 2.1186x; 1.2212x over previous
"""ADMM-net 2D kernel for 8 TRN2 NeuronCores.

Math: in the reference, b stays exactly 0 and every stage is a linear map of
theta, so the whole 9-stage net collapses to theta = y @ M9 where M9 is a
tiny 64x121 matrix computed from Phi and the gammas:

    M_0 = Phi,  M_{k+1} = M_k + (I - M_k Phi^T) S_k Phi,
    S_k = diag(1 / (rm + gamma_k)),  rm = rowwise ||Phi||^2.

On device the recurrence is evaluated in its E-form, which has only one
64x64 matmul + one PSUM->SBUF copy per stage on the critical path:

    G = Phi Phi^T,  Et_0 = I - G,  Et_{k} = (I - S_{k-1} G)^T-form chain
    F~ = sum_k S_k Et_k,   M9 = Phi + F~^T Phi

The big matmul theta = y @ M9 runs in fp16 (y cast during DMA; M9 scaled by
2^-36 so its ~1e14 entries fit fp16; rescaled by 2^36 in the PSUM->SBUF
copies; PSUM accumulation is fp32).  Row-tiles are PE-transposed in PAIRS
([128,128] block -> tiles at psum partitions 0-63 / 64-127) so two row-
packed K=64 matmuls run concurrently in the PE array; the two concurrent
matmuls write DIFFERENT PSUM banks (same-bank concurrent writes fault).

DMA: row-tile c is the strided row set {p*128 + c} (a permutation), making
every partition's DMA data one long contiguous DRAM run.

Sharding: pure data-parallel over the batch dim: 131072 rows -> 8 cores x
16384 rows. No collectives.
"""

import sys

if "/opt/trn_rl_repo" not in sys.path:
    sys.path.insert(0, "/opt/trn_rl_repo")

import numpy as np

B, M, N = 131072, 64, 121
STAGES = 9
NCORES = 8
BS = B // NCORES          # 16384 rows per core
TILES = BS // 128         # 128 row-tiles per core
CHUNK = 16                # row-tiles per DMA chunk
NCHUNKS = TILES // CHUNK
BLOBW = 128 + 64 + N + STAGES + N  # ident | identh bits | phi | gam | phi2s

_cached = {}


def _build_nc():
    from concourse import bacc, mybir, tile

    f32 = mybir.dt.float32
    f16 = mybir.dt.float16
    Alu = mybir.AluOpType
    SCALE = float(2.0 ** 36)

    nc = bacc.Bacc("TRN2", target_bir_lowering=False, debug=False)

    y_d = nc.dram_tensor("y", [BS, M], f32, kind="ExternalInput")
    blob_d = nc.dram_tensor("blob", [128, BLOBW], f32, kind="ExternalInput")
    out_d = nc.dram_tensor("out", [BS, N], f32, kind="ExternalOutput")

    with tile.TileContext(nc) as tc:
        with (
            tc.tile_pool(name="const", bufs=1) as constp,
            tc.tile_pool(name="setup", bufs=2) as setp,
            tc.tile_pool(name="pst", bufs=1, space="PSUM") as pstp,
            tc.tile_pool(name="pch", bufs=2, space="PSUM") as pchp,
            tc.tile_pool(name="ypool", bufs=4) as ypool,
            tc.tile_pool(name="ytsb", bufs=8) as ytsbp,
            tc.tile_pool(name="opool", bufs=2) as opool,
            tc.tile_pool(name="ytps", bufs=3, space="PSUM") as ytpsp,
            tc.tile_pool(name="thps", bufs=2, space="PSUM") as thpsp,
        ):
            # ---- constants / small inputs: ONE packed DMA ----
            blob_sb = constp.tile([128, BLOBW], f32)
            nc.sync.dma_start(blob_sb[:], blob_d[:])
            ident_sb = blob_sb[:, 0:128]
            identh_sb = blob_sb[:, 128:192].bitcast(f16)
            phi_sb = blob_sb[:M, 192 : 192 + N]
            gam_sb = blob_sb[:M, 192 + N : 192 + N + STAGES]
            phi2s_sb = blob_sb[:, 192 + N + STAGES : 192 + N + STAGES + N]
            I64 = ident_sb[:M, :M]

            # ---- setup: s = 1/(rm + gamma)  [64, 9] ----
            sq = setp.tile([M, N], f32, tag="sq")
            nc.vector.tensor_tensor(sq[:], phi_sb, phi_sb, Alu.mult)
            rm = constp.tile([M, 1], f32)
            nc.vector.reduce_sum(rm[:], sq[:], axis=mybir.AxisListType.X)
            rg = setp.tile([M, STAGES], f32, tag="rg")
            nc.vector.tensor_scalar(rg[:], gam_sb, rm[:], None, Alu.add)
            s_sb = constp.tile([M, STAGES], f32)
            nc.vector.reciprocal(s_sb[:], rg[:])

            # ---- setup: G = Phi Phi^T ----
            phiT_ps = pstp.tile([N, M], f32, tag="tp")
            nc.tensor.transpose(phiT_ps[:], phi_sb, I64)
            phiT_sb = constp.tile([N, M], f32)
            nc.vector.tensor_copy(phiT_sb[:], phiT_ps[:])
            g_ps = pchp.tile([M, M], f32, tag="g")
            nc.tensor.matmul(g_ps[:], phiT_sb[:], phiT_sb[:])
            g_sb = constp.tile([M, M], f32)
            nc.vector.tensor_copy(g_sb[:], g_ps[:])

            # off-chain: lhsT_k = I - S_k G for k = 0..7 (row scale + sub)
            lh_all = constp.tile([M, STAGES - 1, M], f32)
            for k in range(STAGES - 1):
                nc.vector.tensor_scalar(
                    lh_all[:, k, :], g_sb[:], s_sb[:, k : k + 1], None, Alu.mult
                )
                nc.vector.tensor_tensor(
                    lh_all[:, k, :], I64, lh_all[:, k, :], Alu.subtract
                )

            # ---- chain: Et_0 = I - G; Et_k = lhsT_{k-1}^T @ Et_{k-1} ----
            et_sb = setp.tile([M, M], f32, tag="et")
            nc.vector.tensor_tensor(et_sb[:], I64, g_ps[:], Alu.subtract)
            facc = setp.tile([M, M], f32, tag="f0")
            nc.vector.tensor_scalar(
                facc[:], et_sb[:], s_sb[:, 0:1], None, Alu.mult
            )
            for k in range(1, STAGES):
                e_ps = pchp.tile([M, M], f32, tag="g")
                nc.tensor.matmul(e_ps[:], lh_all[:, k - 1, :], et_sb[:])
                et_new = setp.tile([M, M], f32, tag="et")
                nc.vector.tensor_copy(et_new[:], e_ps[:])
                et_sb = et_new
                # F~ += S_k Et_k  (off the matmul chain)
                fterm = setp.tile([M, M], f32, tag="ft")
                nc.vector.tensor_scalar(
                    fterm[:], e_ps[:], s_sb[:, k : k + 1], None, Alu.mult
                )
                facc_new = setp.tile([M, M], f32, tag="f0")
                nc.vector.tensor_tensor(facc_new[:], facc[:], fterm[:], Alu.add)
                facc = facc_new

            # ---- M9 (scaled, fp16, stacked on both partition halves) ----
            # F2s = [2^-36 F~ | 2^-36 F~]  ->  m9add = F2s^T Phi = 2^-36 F Phi
            f2s_sb = setp.tile([M, 128], f32, tag="f2")
            nc.vector.tensor_scalar(
                f2s_sb[:, :M], facc[:], 1.0 / SCALE, None, Alu.mult
            )
            nc.vector.tensor_scalar(
                f2s_sb[:, M:], facc[:], 1.0 / SCALE, None, Alu.mult
            )
            m9add_ps = pstp.tile([128, N], f32, tag="tp")
            nc.tensor.matmul(m9add_ps[:], f2s_sb[:], phi_sb)
            m9h_sb = constp.tile([128, N], f16)
            nc.vector.tensor_tensor(
                m9h_sb[:], phi2s_sb, m9add_ps[:], Alu.add
            )

            # ---- main loop: theta = y @ M9, 128-row tiles ----
            y_v = y_d[:].rearrange("(p c) m -> p c m", c=TILES)
            out_v = out_d[:].rearrange("(p c) n -> p c n", c=TILES)
            for c in range(NCHUNKS):
                c0 = c * CHUNK
                y_sb = ypool.tile([128, CHUNK, M], f16, tag="y")
                nc.gpsimd.dma_start(y_sb[:], y_v[:, c0 : c0 + CHUNK, :])
                th_sb = opool.tile([128, CHUNK, N], f32, tag="th")
                for g in range(CHUNK // 8):
                    yt_ps = ytpsp.tile([128, 512], f16, tag="ytp")
                    # One [128,128] transpose per PAIR of row-tiles: tile 2i
                    # lands at psum partitions 0-63, tile 2i+1 at 64-127.
                    for i in range(4):
                        t = g * 8 + 2 * i
                        nc.tensor.transpose(
                            yt_ps[:, i * 128 : (i + 1) * 128],
                            y_sb[:, t : t + 2, :],
                            identh_sb,
                        )
                    yt_sb = ytsbp.tile([128, 512], f16, tag="yts")
                    nc.vector.tensor_copy(yt_sb[:], yt_ps[:])
                    # Adjacent matmuls alternate PE row-groups (concurrent)
                    # and must land in different PSUM banks.
                    thA = thpsp.tile([128, 4, N], f32, tag="thp")
                    thB = thpsp.tile([128, 4, N], f32, tag="thp")
                    for i in range(4):
                        slot = 128 * i
                        nc.tensor.matmul(
                            thA[:, i, :],
                            yt_sb[0:64, slot : slot + 128],
                            m9h_sb[0:64, :],
                            tile_position=(0, 0),
                        )
                        nc.tensor.matmul(
                            thB[:, i, :],
                            yt_sb[64:128, slot : slot + 128],
                            m9h_sb[64:128, :],
                            tile_position=(64, 0),
                        )
                    tbase = g * 8
                    nc.vector.tensor_scalar(
                        th_sb[:, tbase : tbase + 8 : 2, :], thA[:],
                        SCALE, None, Alu.mult,
                    )
                    nc.scalar.mul(
                        th_sb[:, tbase + 1 : tbase + 8 : 2, :], thB[:], SCALE
                    )
                nc.sync.dma_start(out_v[:, c0 : c0 + CHUNK, :], th_sb[:])

    nc.compile()
    return nc


def _get_nc():
    if "nc" not in _cached:
        _cached["nc"] = _build_nc()
    return _cached["nc"]


def kernel(y, Phi, gammas):
    from concourse.bass_utils import run_bass_kernel_spmd

    y = np.ascontiguousarray(y, dtype=np.float32)
    phi = np.asarray(Phi, dtype=np.float32)
    gam = np.asarray(gammas, dtype=np.float32).reshape(1, STAGES)

    blob = np.zeros((128, BLOBW), dtype=np.float32)
    blob[:, 0:128] = np.eye(128, dtype=np.float32)
    blob[:, 128:192] = np.eye(128, dtype=np.float16).view(np.float32)
    blob[:M, 192 : 192 + N] = phi
    blob[:M, 192 + N : 192 + N + STAGES] = np.broadcast_to(gam, (M, STAGES))
    phi2s = (phi * np.float32(2.0 ** -36)).astype(np.float32)
    blob[:M, 192 + N + STAGES :] = phi2s
    blob[M:, 192 + N + STAGES :] = phi2s

    nc = _get_nc()
    in_maps = [
        {
            "y": np.ascontiguousarray(y[i * BS : (i + 1) * BS]),
            "blob": blob,
        }
        for i in range(NCORES)
    ]
    res = run_bass_kernel_spmd(nc, in_maps, core_ids=list(range(NCORES)))
    _cached["last_run"] = res
    return np.concatenate([res.results[i]["out"] for i in range(NCORES)], axis=0)
